# revision 2
# baseline (speedup 1.0000x reference)
"""Conv2D 3x3 (B=32, C=128, H=W=56 -> OC=256) as a Bass/Tile kernel on 8 NeuronCores.

Strategy: data-parallel over batch (4 images per core), W/b replicated.
The conv is computed as 9 shift-matmuls accumulated in PSUM:
  out[oc, h, w] = sum_{kh,kw} W[oc, :, kh, kw] @ x_pad[:, h+kh, w+kw]
with x zero-padded to 58x58 on the host so every shifted window is a clean
strided view of one SBUF tile. Contraction dim C=128 sits on partitions,
OC=256 is two 128-row output tiles, and the 56x56 output plane is processed
in 7 blocks of 8 rows (N = 8*56 = 448 <= 512, one PSUM bank).

Perf notes (measured on hw):
 - The matmul stream runs at the warm f16 roofline (freedim/2.4GHz spacing,
   LDWEIGHTS hidden by the PE background weight buffer), so the wins are in
   the edges: no explicit HAM warm-up (the real stream warms the clock gate
   while the lead-in DMAs land), lead-in DMAs split across the SP and ACT
   hw-DGE queues finest-first in matmul need-order, and matmul columns that
   would only multiply the zero padding are trimmed via 3D sub-views of the
   PSUM tile (center tap goes first with start=True so every PSUM element
   the drain reads is written).
 - The last output-row block is split 6+2 so the final ACT+DMA tail is
   short.

matmul dtype: float16 by default (full PE rate with fast weight load via
FWL; ~3e-4 rel err vs the fp32 reference given this problem's small dynamic
range). Set CONV_MM_DTYPE=f32r (~1.5e-4 err, slower), bf16, or f32 to
switch.
"""

import os

import numpy as np

import concourse.bacc as bacc
import concourse.mybir as mybir
import concourse.tile as tile
from concourse import bass_utils

B, C, H, W_SP = 32, 128, 56, 56
OC, KH, KW = 256, 3, 3
N_CORES = 8
B_PER = B // N_CORES            # 4 images per core
HP, WP = H + 2, W_SP + 2        # zero-padded spatial dims (58x58)
HWP = HP * WP                   # 3364
HWO = H * W_SP                  # 3136
ROWS_PER_TILE = 8               # output rows per matmul tile
N_TILE = ROWS_PER_TILE * W_SP   # 448 (<=512: one PSUM bank)
N_NT = H // ROWS_PER_TILE       # 7
OC_TILES = OC // 128            # 2

# center tap first (always full-region: start=True must cover everything the
# PSUM drain reads), then the kh=1 row, then top/bottom rows -- this is also
# the order the lead-in weight DMAs land in.
TAPS = [(1, 1), (1, 0), (1, 2), (0, 0), (0, 1), (0, 2), (2, 0), (2, 1), (2, 2)]

_NC_CACHE: dict[str, object] = {}


def _mm_mode() -> str:
    return os.environ.get("CONV_MM_DTYPE", "f16")


def _build_nc(mode: str):
    in_dt = {
        "bf16": mybir.dt.bfloat16,
        "f16": mybir.dt.float16,
        "f32r": mybir.dt.float32r,
        "f32": mybir.dt.float32,
    }[mode]
    nc = bacc.Bacc(
        "TRN2",
        target_bir_lowering=False,
        debug=False,
        enable_asserts=False,
        num_devices=N_CORES,
    )
    xp = nc.dram_tensor("xp", [B_PER, C, HWP], in_dt, kind="ExternalInput").ap()
    wt = nc.dram_tensor("wt", [C, KH * KW * OC], in_dt, kind="ExternalInput").ap()
    bias = nc.dram_tensor(
        "bias", [128, OC_TILES], mybir.dt.float32, kind="ExternalInput"
    ).ap()
    out = nc.dram_tensor(
        "out", [B_PER, OC, HWO], mybir.dt.float32, kind="ExternalOutput"
    ).ap()

    CHUNK_ROWS = ROWS_PER_TILE + KH - 1  # 10 padded rows per chunk (2-row halo)

    with tile.TileContext(nc) as tc:
        with (
            tc.tile_pool(name="xin", bufs=16) as xpool,
            tc.tile_pool(name="wpool", bufs=1) as wpool,
            tc.tile_pool(name="bpool", bufs=1) as bpool,
            tc.tile_pool(name="opool", bufs=4) as opool,
            tc.tile_pool(name="psum", bufs=4, space="PSUM") as pspool,
        ):
            # Lead-in DMAs, finest first in matmul need-order, split across
            # the two hw-DGE queues (SP + ACT) so issue cost (~600ns each)
            # doesn't serialize.  Need times assume the cold (1.2 GHz) PE:
            # MM k of the first group starts ~8.3us + 0.37us*k.
            #   SP : xc0 (chunk 0),      w[kh=0 row, taps 3..5 here]
            #   ACT: w(1,1), w(1,0), w(1,2), w[kh=2 row]
            # Bias rides GpSimd (software DGE, off both hw queues).
            wsb = wpool.tile([C, KH * KW, OC], in_dt, tag="wsb")
            wtv = wt.rearrange("c (k m) -> c k m", m=OC)
            xv0 = xp[0].rearrange("c (h w) -> c h w", w=WP)
            xc0 = xpool.tile([C, CHUNK_ROWS, WP], in_dt, tag="xc")
            nc.sync.dma_start(xc0[:], xv0[:, :CHUNK_ROWS, :])
            nc.scalar.dma_start(wsb[:, 4, :], wtv[:, 4, :])  # (1,1)
            nc.scalar.dma_start(wsb[:, 3, :], wtv[:, 3, :])  # (1,0)
            nc.scalar.dma_start(wsb[:, 5, :], wtv[:, 5, :])  # (1,2)
            nc.sync.dma_start(wsb[:, 0:3, :], wtv[:, 0:3, :])  # kh=0 row
            nc.scalar.dma_start(wsb[:, 6:9, :], wtv[:, 6:9, :])  # kh=2 row
            bsb = bpool.tile([128, OC_TILES], mybir.dt.float32, tag="bsb")
            nc.gpsimd.dma_start(bsb[:], bias[:])

            for img in range(B_PER):
                xv = xp[img].rearrange("c (h w) -> c h w", w=WP)
                for nt in range(N_NT):
                    blk0 = nt * ROWS_PER_TILE
                    if img == 0 and nt == 0:
                        xc = xc0
                    else:
                        xc = xpool.tile([C, CHUNK_ROWS, WP], in_dt, tag="xc")
                        nc.sync.dma_start(
                            xc[:], xv[:, blk0 : blk0 + CHUNK_ROWS, :]
                        )
                    for oc_t in range(OC_TILES):
                        # the very last group is split 6+2 so the final
                        # ACT+DMA drain after the last matmul is short
                        is_last = (
                            img == B_PER - 1
                            and nt == N_NT - 1
                            and oc_t == OC_TILES - 1
                        )
                        subs = [(0, 6), (6, 2)] if is_last else [(0, ROWS_PER_TILE)]
                        for sr, nr in subs:
                            n_free = nr * W_SP
                            ps = pspool.tile(
                                [128, ROWS_PER_TILE, W_SP],
                                mybir.dt.float32,
                                tag="ps",
                            )
                            n_taps = len(TAPS)
                            for ki, (kh, kw) in enumerate(TAPS):
                                # trim rows/cols whose input is all zero
                                # padding; the center tap (ki==0) is always
                                # full so start=True covers the drain region
                                r0, r1 = sr, sr + nr
                                if nt == 0 and kh == 0:
                                    r0 = max(r0, 1)
                                if nt == N_NT - 1 and kh == 2:
                                    r1 = min(r1, ROWS_PER_TILE - 1)
                                c0, c1 = 0, W_SP
                                if kw == 0:
                                    c0 = 1
                                elif kw == 2:
                                    c1 = W_SP - 1
                                rhs = xc[:, kh + r0 : kh + r1, kw + c0 : kw + c1]
                                lhsT = wsb[
                                    :,
                                    kh * KW + kw,
                                    oc_t * 128 : (oc_t + 1) * 128,
                                ]
                                nc.tensor.matmul(
                                    ps[:, r0 - sr : r1 - sr, c0:c1],
                                    lhsT,
                                    rhs,
                                    start=(ki == 0),
                                    stop=(ki == n_taps - 1),
                                )
                            ot = opool.tile(
                                [128, N_TILE], mybir.dt.float32, tag="ot"
                            )
                            psf = ps[:, :nr, :].rearrange("p r c -> p (r c)")
                            nc.scalar.activation(
                                ot[:, :n_free],
                                psf,
                                mybir.ActivationFunctionType.Identity,
                                bias=bsb[:, oc_t : oc_t + 1],
                            )
                            col0 = nt * N_TILE + sr * W_SP
                            nc.sync.dma_start(
                                out[
                                    img,
                                    oc_t * 128 : (oc_t + 1) * 128,
                                    col0 : col0 + n_free,
                                ],
                                ot[:, :n_free],
                            )
    nc.compile()
    return nc


def _get_nc(mode: str):
    nc = _NC_CACHE.get(mode)
    if nc is None:
        nc = _build_nc(mode)
        _NC_CACHE[mode] = nc
    return nc


def kernel(x: np.ndarray, W: np.ndarray, b: np.ndarray) -> np.ndarray:
    mode = _mm_mode()
    x = np.asarray(x, dtype=np.float32)
    W = np.asarray(W, dtype=np.float32)
    b = np.asarray(b, dtype=np.float32)

    if mode == "bf16":
        import ml_dtypes

        in_np_dt = ml_dtypes.bfloat16
    elif mode == "f16":
        in_np_dt = np.float16
    else:
        in_np_dt = np.float32

    # Host-side layout prep: zero-pad x spatially, put the conv taps of W
    # into [tap, C, OC] (lhsT layout), stripe bias to [128, OC_TILES].
    xp = np.zeros((B, C, HP, WP), dtype=in_np_dt)
    xp[:, :, 1:-1, 1:-1] = x
    xp = xp.reshape(N_CORES, B_PER, C, HWP)
    # wt[c, k*OC + oc] = W[oc, c*9 + k]  (lhsT tap blocks, contiguous per c)
    wt = np.ascontiguousarray(
        W.reshape(OC, C, KH * KW).transpose(1, 2, 0).reshape(C, KH * KW * OC)
    ).astype(in_np_dt)
    bias = np.ascontiguousarray(b.reshape(OC_TILES, 128).T).astype(np.float32)

    nc = _get_nc(mode)
    in_maps = [
        {"xp": np.ascontiguousarray(xp[i]), "wt": wt, "bias": bias}
        for i in range(N_CORES)
    ]
    trace = os.environ.get("CONV_TRACE", "") not in ("", "0")
    try:
        res = bass_utils.run_bass_kernel_spmd(
            nc,
            in_maps,
            core_ids=list(range(N_CORES)),
            trace=trace,
        )
    except Exception:
        # transient device wedges (NRT_EXEC_UNIT_UNRECOVERABLE) have been
        # observed once; a fresh dispatch usually recovers
        import time

        time.sleep(2.0)
        res = bass_utils.run_bass_kernel_spmd(
            nc,
            in_maps,
            core_ids=list(range(N_CORES)),
            trace=trace,
        )
    kernel._last_results = res  # for test harness introspection
    out = np.stack([res.results[i]["out"] for i in range(N_CORES)])
    return out.reshape(B, OC, H, W_SP)


# revision 4
# speedup vs baseline: 1.0256x; 1.0256x over previous
"""Conv2D 3x3 (B=32, C=128, H=W=56 -> OC=256) as a Bass/Tile kernel on 8 NeuronCores.

Strategy: data-parallel over batch (4 images per core), W/b replicated.
The conv is computed as 9 shift-matmuls accumulated in PSUM:
  out[oc, h, w] = sum_{kh,kw} W[oc, :, kh, kw] @ x_pad[:, h+kh, w+kw]
with x zero-padded to 58x58 on the host so every shifted window is a clean
strided view of one SBUF tile. Contraction dim C=128 sits on partitions,
OC=256 is two 128-row output tiles, and the 56x56 output plane is processed
in 7 blocks of 8 rows (N = 8*56 = 448 <= 512, one PSUM bank).

Perf notes (measured on hw):
 - The matmul stream runs at the warm f16 roofline (freedim/2.4GHz spacing,
   LDWEIGHTS hidden by the PE background weight buffer), so the wins are in
   the edges: matmul columns that would only multiply the zero padding are
   trimmed via 3D sub-views of the PSUM tile (center tap goes first with
   start=True so every PSUM element the drain reads is written), and the
   last output-row block is split 6+2 so the final ACT+DMA tail is short.
 - Keep the HAM warm-up burn + single-Sync-queue lead-in: measured, the
   first input DMAs only land ~10.5-12us in (ring spin-up dominates), and
   removing the warm-up makes the first ~13 real matmuls run at the cold
   1.2 GHz clock with DMA stalls resetting the HAM busy window -- a net
   ~2us loss vs burning dummy matmuls while the DMAs fly.

matmul dtype: float16 by default (full PE rate with fast weight load via
FWL; ~3e-4 rel err vs the fp32 reference given this problem's small dynamic
range). Set CONV_MM_DTYPE=f32r (~1.5e-4 err, slower), bf16, or f32 to
switch.
"""

import os

import numpy as np

import concourse.bacc as bacc
import concourse.mybir as mybir
import concourse.tile as tile
from concourse import bass_utils

B, C, H, W_SP = 32, 128, 56, 56
OC, KH, KW = 256, 3, 3
N_CORES = 8
B_PER = B // N_CORES            # 4 images per core
HP, WP = H + 2, W_SP + 2        # zero-padded spatial dims (58x58)
HWP = HP * WP                   # 3364
HWO = H * W_SP                  # 3136
ROWS_PER_TILE = 8               # output rows per matmul tile
N_TILE = ROWS_PER_TILE * W_SP   # 448 (<=512: one PSUM bank)
N_NT = H // ROWS_PER_TILE       # 7
OC_TILES = OC // 128            # 2

# center tap first (always full-region: start=True must cover everything the
# PSUM drain reads), then the kh=1 row, then top/bottom rows -- this is also
# the order the lead-in weight DMAs land in.
TAPS = [(1, 1), (1, 0), (1, 2), (0, 0), (0, 1), (0, 2), (2, 0), (2, 1), (2, 2)]

_NC_CACHE: dict[str, object] = {}


def _mm_mode() -> str:
    return os.environ.get("CONV_MM_DTYPE", "f16")


def _build_nc(mode: str):
    in_dt = {
        "bf16": mybir.dt.bfloat16,
        "f16": mybir.dt.float16,
        "f32r": mybir.dt.float32r,
        "f32": mybir.dt.float32,
    }[mode]
    nc = bacc.Bacc(
        "TRN2",
        target_bir_lowering=False,
        debug=False,
        enable_asserts=False,
        num_devices=N_CORES,
    )
    xp = nc.dram_tensor("xp", [B_PER, C, HWP], in_dt, kind="ExternalInput").ap()
    wt = nc.dram_tensor("wt", [C, KH * KW * OC], in_dt, kind="ExternalInput").ap()
    bias = nc.dram_tensor(
        "bias", [128, OC_TILES], mybir.dt.float32, kind="ExternalInput"
    ).ap()
    out = nc.dram_tensor(
        "out", [B_PER, OC, HWO], mybir.dt.float32, kind="ExternalOutput"
    ).ap()

    CHUNK_ROWS = ROWS_PER_TILE + KH - 1  # 10 padded rows per chunk (2-row halo)

    with tile.TileContext(nc) as tc:
        with (
            tc.tile_pool(name="xin", bufs=16) as xpool,
            tc.tile_pool(name="wpool", bufs=1) as wpool,
            tc.tile_pool(name="bpool", bufs=1) as bpool,
            tc.tile_pool(name="opool", bufs=4) as opool,
            tc.tile_pool(name="psum", bufs=4, space="PSUM") as pspool,
        ):
            # HAM warm-up: the PE clock-gate needs ~3.4us of sustained matmul
            # activity to lift to 2.4 GHz. Burn dummy matmuls on a zeroed tile
            # while the first DMAs are still in flight so the real stream
            # starts warm.
            wu = wpool.tile([C, 512], in_dt, tag="wu")
            nc.gpsimd.memset(wu[:], 0.0)
            psw = pspool.tile([128, ROWS_PER_TILE, W_SP], mybir.dt.float32, tag="ps")
            for i in range(7):
                nc.tensor.matmul(
                    psw[:, :, :].rearrange("p r c -> p (r c)"),
                    wu[:, :128],
                    wu[:, :N_TILE],
                    start=(i == 0),
                    stop=(i == 6),
                )

            # lead-in DMAs, finest first in matmul need-order (the k-th
            # matmul of the first PSUM group needs tap TAPS[k] and chunk 0).
            # Bias rides GpSimd (off the critical Sync issue queue).
            wsb = wpool.tile([C, KH * KW, OC], in_dt, tag="wsb")
            wtv = wt.rearrange("c (k m) -> c k m", m=OC)
            xv0 = xp[0].rearrange("c (h w) -> c h w", w=WP)
            nc.sync.dma_start(wsb[:, 4, :], wtv[:, 4, :])  # (1,1) center
            xc0 = xpool.tile([C, CHUNK_ROWS, WP], in_dt, tag="xc")
            nc.sync.dma_start(xc0[:], xv0[:, :CHUNK_ROWS, :])
            nc.sync.dma_start(wsb[:, 3, :], wtv[:, 3, :])  # (1,0)
            nc.sync.dma_start(wsb[:, 5, :], wtv[:, 5, :])  # (1,2)
            nc.sync.dma_start(wsb[:, 0:3, :], wtv[:, 0:3, :])  # kh=0 row
            nc.sync.dma_start(wsb[:, 6:9, :], wtv[:, 6:9, :])  # kh=2 row
            bsb = bpool.tile([128, OC_TILES], mybir.dt.float32, tag="bsb")
            nc.gpsimd.dma_start(bsb[:], bias[:])

            for img in range(B_PER):
                xv = xp[img].rearrange("c (h w) -> c h w", w=WP)
                for nt in range(N_NT):
                    blk0 = nt * ROWS_PER_TILE
                    if img == 0 and nt == 0:
                        xc = xc0
                    else:
                        xc = xpool.tile([C, CHUNK_ROWS, WP], in_dt, tag="xc")
                        nc.sync.dma_start(
                            xc[:], xv[:, blk0 : blk0 + CHUNK_ROWS, :]
                        )
                    for oc_t in range(OC_TILES):
                        # the very last group is split 6+2 so the final
                        # ACT+DMA drain after the last matmul is short
                        is_last = (
                            img == B_PER - 1
                            and nt == N_NT - 1
                            and oc_t == OC_TILES - 1
                        )
                        subs = [(0, 6), (6, 2)] if is_last else [(0, ROWS_PER_TILE)]
                        for sr, nr in subs:
                            n_free = nr * W_SP
                            ps = pspool.tile(
                                [128, ROWS_PER_TILE, W_SP],
                                mybir.dt.float32,
                                tag="ps",
                            )
                            n_taps = len(TAPS)
                            for ki, (kh, kw) in enumerate(TAPS):
                                # trim rows/cols whose input is all zero
                                # padding; the center tap (ki==0) is always
                                # full so start=True covers the drain region
                                r0, r1 = sr, sr + nr
                                if nt == 0 and kh == 0:
                                    r0 = max(r0, 1)
                                if nt == N_NT - 1 and kh == 2:
                                    r1 = min(r1, ROWS_PER_TILE - 1)
                                c0, c1 = 0, W_SP
                                if kw == 0:
                                    c0 = 1
                                elif kw == 2:
                                    c1 = W_SP - 1
                                rhs = xc[:, kh + r0 : kh + r1, kw + c0 : kw + c1]
                                lhsT = wsb[
                                    :,
                                    kh * KW + kw,
                                    oc_t * 128 : (oc_t + 1) * 128,
                                ]
                                nc.tensor.matmul(
                                    ps[:, r0 - sr : r1 - sr, c0:c1],
                                    lhsT,
                                    rhs,
                                    start=(ki == 0),
                                    stop=(ki == n_taps - 1),
                                )
                            ot = opool.tile(
                                [128, N_TILE], mybir.dt.float32, tag="ot"
                            )
                            psf = ps[:, :nr, :].rearrange("p r c -> p (r c)")
                            nc.scalar.activation(
                                ot[:, :n_free],
                                psf,
                                mybir.ActivationFunctionType.Identity,
                                bias=bsb[:, oc_t : oc_t + 1],
                            )
                            col0 = nt * N_TILE + sr * W_SP
                            nc.sync.dma_start(
                                out[
                                    img,
                                    oc_t * 128 : (oc_t + 1) * 128,
                                    col0 : col0 + n_free,
                                ],
                                ot[:, :n_free],
                            )
    nc.compile()
    return nc


def _get_nc(mode: str):
    nc = _NC_CACHE.get(mode)
    if nc is None:
        nc = _build_nc(mode)
        _NC_CACHE[mode] = nc
    return nc


def kernel(x: np.ndarray, W: np.ndarray, b: np.ndarray) -> np.ndarray:
    mode = _mm_mode()
    x = np.asarray(x, dtype=np.float32)
    W = np.asarray(W, dtype=np.float32)
    b = np.asarray(b, dtype=np.float32)

    if mode == "bf16":
        import ml_dtypes

        in_np_dt = ml_dtypes.bfloat16
    elif mode == "f16":
        in_np_dt = np.float16
    else:
        in_np_dt = np.float32

    # Host-side layout prep: zero-pad x spatially, put the conv taps of W
    # into [tap, C, OC] (lhsT layout), stripe bias to [128, OC_TILES].
    xp = np.zeros((B, C, HP, WP), dtype=in_np_dt)
    xp[:, :, 1:-1, 1:-1] = x
    xp = xp.reshape(N_CORES, B_PER, C, HWP)
    # wt[c, k*OC + oc] = W[oc, c*9 + k]  (lhsT tap blocks, contiguous per c)
    wt = np.ascontiguousarray(
        W.reshape(OC, C, KH * KW).transpose(1, 2, 0).reshape(C, KH * KW * OC)
    ).astype(in_np_dt)
    bias = np.ascontiguousarray(b.reshape(OC_TILES, 128).T).astype(np.float32)

    nc = _get_nc(mode)
    in_maps = [
        {"xp": np.ascontiguousarray(xp[i]), "wt": wt, "bias": bias}
        for i in range(N_CORES)
    ]
    trace = os.environ.get("CONV_TRACE", "") not in ("", "0")
    try:
        res = bass_utils.run_bass_kernel_spmd(
            nc,
            in_maps,
            core_ids=list(range(N_CORES)),
            trace=trace,
        )
    except Exception:
        # transient device wedges (NRT_EXEC_UNIT_UNRECOVERABLE) have been
        # observed once; a fresh dispatch usually recovers
        import time

        time.sleep(2.0)
        res = bass_utils.run_bass_kernel_spmd(
            nc,
            in_maps,
            core_ids=list(range(N_CORES)),
            trace=trace,
        )
    kernel._last_results = res  # for test harness introspection
    out = np.stack([res.results[i]["out"] for i in range(N_CORES)])
    return out.reshape(B, OC, H, W_SP)
